# revision 1
# baseline (speedup 1.0000x reference)
"""Trainium2 Bass kernel for BiaffinePairing.

Computes S = (T @ W) @ A^T + T @ U[:H] + (A @ U[H:]).T + b  -> [4096, 4096] f32.

Strategy (8 NeuronCores, data-parallel over T's row dim n):
  - Host-side layout prep only (no math): transpose T and A so the
    contraction dim H=1024 lies on SBUF partitions; shard T^T's columns
    (the n dim) 8 ways; replicate A^T, W, and the U halves. Matmul inputs
    are pre-cast to fp16 on the host (the DMA streams halve; measured
    3.7e-4 relative error end-to-end vs the fp32 reference).
  - Per core: mm1 computes TWt[h_out, n] = (T_shard @ W)^T accumulating in
    PSUM; the rank-1 term 1_n (x) (A @ u_a)^T folds in by adding u_a[h] as
    a per-partition bias on mm1's PSUM->SBUF copy (since
    (TW + 1 (x) u_a^T) @ A^T = TW@A^T + 1 (x) (A@u_a)^T).
  - tvec[n] = T_shard @ u_t + b via tiny matmuls; added as the per-partition
    bias on mm2's PSUM->SBUF copies.
  - mm2 computes S_shard[n, m] = sum_k TWt[k]^T @ At[k] over m-chunks.

Schedule notes (why the structure looks the way it does):
  - Dummy warmup matmuls run during the ~7us framework preamble so the PE
    HAM clock-gate reaches 8/8 before real work.
  - mm1 is k-outer over 4 PSUM banks in two ho-half passes: each k step
    needs only W/tT k-tile k, so the PE chases the load DMAs.
  - Load DMAs alternate between the two HWDGE FIFOs (sync/scalar) to halve
    the ~650ns-per-dma_start issue serialization; out stores also go on
    scalar so they never head-of-line-block the at-chunk loads on sync.
"""

import numpy as np

import concourse.bacc as bacc
import concourse.mybir as mybir
from concourse.tile import TileContext
from concourse.bass_utils import run_bass_kernel_spmd

H = 1024          # hidden dim (contraction)
N_TOT = 4096      # rows of target_spans
M_TOT = 4096      # rows of argument_spans
N_CORES = 8
NSH = N_TOT // N_CORES   # 512 n rows per core
KT = H // 128            # 8 contraction k-tiles
NI = NSH // 128          # 4 n-tiles of 128 per core
MCH = 1024               # m-chunk width
MC = M_TOT // MCH        # 4 m-chunks
MH = MCH // 512          # 512-wide psum sub-slices per chunk

F32 = mybir.dt.float32
F16 = mybir.dt.float16

_NC_CACHE = {}


def _build(b_val: float, warm: int = 16, late_uv: bool = False):
    nc = bacc.Bacc("TRN2", target_bir_lowering=False, debug=False,
                   num_devices=N_CORES)

    tT = nc.dram_tensor("tT", [H, NSH], F16, kind="ExternalInput")
    aT = nc.dram_tensor("aT", [H, M_TOT], F16, kind="ExternalInput")
    W = nc.dram_tensor("W", [H, H], F16, kind="ExternalInput")
    # ut comes in as [H, 2] (two identical columns -> a 2-wide moving
    # operand; both result columns equal tvec).
    ut = nc.dram_tensor("ut", [H, 2], F16, kind="ExternalInput")
    ua = nc.dram_tensor("ua", [H, 1], F32, kind="ExternalInput")
    out = nc.dram_tensor("out", [NSH, M_TOT], F32, kind="ExternalOutput")

    # DRAM views with the k-tile index split out: row kt*128 + p.
    tT_v = tT.rearrange("(kt p) n -> p kt n", p=128)
    aT_v = aT.rearrange("(kt p) m -> p kt m", p=128)
    W_v = W.rearrange("(kt p) f -> p kt f", p=128)
    ut_v = ut.rearrange("(kt p) two -> p kt two", p=128)
    ua_v = ua.rearrange("(kt p) one -> p (kt one)", p=128)

    with TileContext(nc) as tc:
        with (
            tc.tile_pool(name="const", bufs=1) as cpool,
            tc.tile_pool(name="achunk", bufs=4) as apool,
            tc.tile_pool(name="outbuf", bufs=6) as opool,
            tc.tile_pool(name="ps1", bufs=1, space="PSUM") as ps1pool,
            tc.tile_pool(name="ps2", bufs=4, space="PSUM") as ps2pool,
        ):
            # ---- constant loads, per k-tile, alternating FIFOs ----
            w_sb = cpool.tile([128, KT, H], F16, tag="w")
            tT_sb = cpool.tile([128, KT, NSH], F16, tag="tT")
            ua_sb = cpool.tile([128, KT], F32, tag="ua")
            ut_sb = cpool.tile([128, KT, 2], F16, tag="ut")
            if not late_uv:
                nc.sync.dma_start(out=ut_sb[:], in_=ut_v[:])
                nc.scalar.dma_start(out=ua_sb[:], in_=ua_v[:])
            for k in range(KT):
                eng_w = nc.sync if k % 2 == 0 else nc.scalar
                eng_t = nc.scalar if k % 2 == 0 else nc.sync
                eng_w.dma_start(out=w_sb[:, k, :], in_=W_v[:, k, :])
                eng_t.dma_start(out=tT_sb[:, k, :], in_=tT_v[:, k, :])
            if late_uv:
                # ut/ua aren't needed until mm1's copy-out (~14us in); issuing
                # them after the W/tT tiles keeps the first k-tiles' issue
                # slots on the critical path.
                nc.sync.dma_start(out=ut_sb[:], in_=ut_v[:])
                nc.scalar.dma_start(out=ua_sb[:], in_=ua_v[:])

            # ---- PE warmup: the first ~7us are framework preamble + DMA
            # ramp with the PE idle, which leaves the HAM clock-gate at
            # K=4/8 (half clock) well into mm1. Dummy matmuls on zeroed
            # tiles trip the HAM busy-window during that dead time so mm1
            # runs at full clock. ----
            warm_w = cpool.tile([128, 128], F16, tag="warm_w")
            warm_in = cpool.tile([128, 512], F16, tag="warm_in")
            nc.vector.memset(warm_w[:], 0.0)
            nc.vector.memset(warm_in[:], 0.0)
            wps = ps1pool.tile([128, NSH], F32, tag="ps1_0", name="wps")
            for _ in range(warm):
                nc.tensor.matmul(wps[:], warm_w[:], warm_in[:],
                                 start=True, stop=True)

            # ---- mm1: TWt[h_out, n] = (T @ W)^T, + u_a bias on copy-out.
            # Two ho-half passes, k-outer over 4 PSUM banks each: a k step
            # only needs the W/tT k-tile k, so the PE starts as soon as the
            # first pair of DMAs lands; pass B reuses the resident W. ----
            twt_sb = cpool.tile([128, KT, NSH], F16, tag="twt")
            ps1 = [ps1pool.tile([128, NSH], F32, tag=f"ps1_{j}",
                                name=f"ps1_{j}")
                   for j in range(4)]
            for half in range(2):
                for k in range(KT):
                    for j in range(4):
                        ho = half * 4 + j
                        nc.tensor.matmul(
                            ps1[j][:],
                            w_sb[:, k, ho * 128:(ho + 1) * 128],
                            tT_sb[:, k, :],
                            start=(k == 0),
                            stop=(k == KT - 1),
                        )
                for j in range(4):
                    ho = half * 4 + j
                    # TWt[ho] = psum + u_a[ho-tile] (per-partition bias),
                    # cast to fp16 for mm2. Copies land after the final
                    # k row; alternate DVE/ACT to halve the serial latency
                    # gating mm2's start.
                    if j % 2 == 0:
                        nc.vector.tensor_scalar_add(
                            out=twt_sb[:, ho, :], in0=ps1[j][:],
                            scalar1=ua_sb[:, ho:ho + 1],
                        )
                    else:
                        nc.scalar.activation(
                            out=twt_sb[:, ho, :], in_=ps1[j][:],
                            func=mybir.ActivationFunctionType.Identity,
                            bias=ua_sb[:, ho:ho + 1],
                        )


            # ---- tvec[n] = T @ u_t + b: 32 tiny matmuls (ut is the 2-wide
            # moving operand; psum column 0 is tvec). ----
            tvec_sb = cpool.tile([128, NI], F32, tag="tvec")
            for ni in range(NI):
                psv = ps2pool.tile([128, 2], F32, tag="ps", name="psv")
                for k in range(KT):
                    nc.tensor.matmul(
                        psv[:],
                        tT_sb[:, k, ni * 128:(ni + 1) * 128],
                        ut_sb[:, k, :],
                        start=(k == 0),
                        stop=(k == KT - 1),
                    )
                nc.scalar.activation(
                    out=tvec_sb[:, ni:ni + 1], in_=psv[:, 0:1],
                    func=mybir.ActivationFunctionType.Identity,
                    bias=float(b_val),
                )

            # ---- mm2: S[n, m] = sum_k TWt[k]^T @ At[k], + tvec bias ----
            for c in range(MC):
                at_sb = apool.tile([128, KT, MCH], F16, tag="at")
                nc.sync.dma_start(
                    out=at_sb[:],
                    in_=aT_v[:, :, c * MCH:(c + 1) * MCH],
                )
                for ni in range(NI):
                    for h in range(MH):
                        ps = ps2pool.tile([128, 512], F32, tag="ps", name="ps")
                        for k in range(KT):
                            nc.tensor.matmul(
                                ps[:],
                                twt_sb[:, k, ni * 128:(ni + 1) * 128],
                                at_sb[:, k, h * 512:(h + 1) * 512],
                                start=(k == 0),
                                stop=(k == KT - 1),
                            )
                        o_sb = opool.tile([128, 512], F32, tag="o")
                        nc.vector.tensor_scalar_add(
                            out=o_sb[:], in0=ps[:],
                            scalar1=tvec_sb[:, ni:ni + 1],
                        )
                        # Stores go on the scalar HWDGE FIFO so they never
                        # head-of-line-block the at-loads on the sync FIFO.
                        nc.scalar.dma_start(
                            out=out[ni * 128:(ni + 1) * 128,
                                    c * MCH + h * 512:c * MCH + (h + 1) * 512],
                            in_=o_sb[:],
                        )

    nc.compile()
    return nc


def _get_nc(b_val: float):
    key = float(b_val)
    if key not in _NC_CACHE:
        _NC_CACHE[key] = _build(key)
    return _NC_CACHE[key]


def make_in_maps(target_spans, argument_spans, W, U, b):
    """Host-side layout prep: shard/transpose/cast the full inputs into the
    per-core input maps. Returns (in_maps, b_val)."""
    target_spans = np.asarray(target_spans, dtype=np.float32)
    argument_spans = np.asarray(argument_spans, dtype=np.float32)
    W = np.ascontiguousarray(np.asarray(W, dtype=np.float16))
    U = np.asarray(U, dtype=np.float32).reshape(2 * H, 1)
    b_val = float(np.asarray(b).reshape(-1)[0])

    tT = np.ascontiguousarray(target_spans.T.astype(np.float16))  # [H, N_TOT]
    aT = np.ascontiguousarray(argument_spans.T.astype(np.float16))  # [H, M_TOT]
    ut = np.ascontiguousarray(
        np.repeat(U[:H], 2, axis=1).astype(np.float16))  # [H, 2]
    ua = np.ascontiguousarray(U[H:])

    in_maps = [
        {
            "tT": np.ascontiguousarray(tT[:, i * NSH:(i + 1) * NSH]),
            "aT": aT,
            "W": W,
            "ut": ut,
            "ua": ua,
        }
        for i in range(N_CORES)
    ]
    return in_maps, b_val


def kernel(target_spans, argument_spans, W, U, b):
    in_maps, b_val = make_in_maps(target_spans, argument_spans, W, U, b)
    nc = _get_nc(b_val)
    res = run_bass_kernel_spmd(nc, in_maps, core_ids=list(range(N_CORES)))
    out = np.concatenate(
        [res.results[i]["out"] for i in range(N_CORES)], axis=0
    )
    return out.astype(np.float32, copy=False)



# revision 2
# speedup vs baseline: 1.0416x; 1.0416x over previous
"""Trainium2 Bass kernel for BiaffinePairing.

Computes S = (T @ W) @ A^T + T @ U[:H] + (A @ U[H:]).T + b  -> [4096, 4096] f32.

Strategy (8 NeuronCores, data-parallel over T's row dim n):
  - Host-side layout prep only (no math): transpose T and A so the
    contraction dim H=1024 lies on SBUF partitions; shard T^T's columns
    (the n dim) 8 ways; replicate A^T, W, and the U halves. Matmul inputs
    are pre-cast to fp16 on the host (the DMA streams halve; ~4e-4
    relative error end-to-end vs the fp32 reference).
  - Per core: mm1 computes TWt[h_out, n] = (T_shard @ W)^T accumulating in
    PSUM; the rank-1 term 1_n (x) (A @ u_a)^T folds in by adding u_a[h] as
    a per-partition bias on mm1's PSUM->SBUF copy (since
    (TW + 1 (x) u_a^T) @ A^T = TW@A^T + 1 (x) (A@u_a)^T).
  - tvec[n] = T_shard @ u_t + b via tiny matmuls; added as the per-partition
    bias on mm2's PSUM->SBUF copies.
  - mm2 computes S_shard[n, m] = sum_k TWt[k]^T @ At[k] over m-chunks.

Schedule notes (why the structure looks the way it is):
  - The PE stream rate is the wall: 320 N=512 matmuls run back-to-back at
    512 cycles each (s2s 216 ns at 2.4 GHz, 259 ns when the package power
    manager drops the core to 2.0 GHz under sustained 8-core load). The
    remaining optimization surface is schedule fat around that stream.
  - The first two HWDGE issue slots carry W k0 / tT k0 so mm1's first
    matmul can start ~1.1 us after the framework preamble ends; ut/ua and
    the first A-chunk go on the GpSimd SWDGE queue so they never consume
    sync/scalar issue slots ahead of W/tT k-tiles.
  - A short warmup burst on memset tiles keeps the PE HAM activity window
    busy from the earliest instant so the clock gate reaches 8/8 (full
    clock) ~3.4 us later, while mm1 is already streaming.
  - mm1 is k-outer over 4 PSUM banks in two ho-half passes: each k step
    needs only W/tT k-tile k, so the PE chases the load DMAs.
  - Outputs are stored as fp16 (upcast to f32 on host): halves the store
    bytes and the end-of-kernel drain. Store dma_starts alternate between
    the sync and scalar HWDGE FIFOs to halve issue serialization.
"""

import numpy as np

import concourse.bacc as bacc
import concourse.mybir as mybir
from concourse.tile import TileContext
from concourse.bass_utils import run_bass_kernel_spmd

H = 1024          # hidden dim (contraction)
N_TOT = 4096      # rows of target_spans
M_TOT = 4096      # rows of argument_spans
N_CORES = 8
NSH = N_TOT // N_CORES   # 512 n rows per core
KT = H // 128            # 8 contraction k-tiles
NI = NSH // 128          # 4 n-tiles of 128 per core
MCH = 1024               # m-chunk width
MC = M_TOT // MCH        # 4 m-chunks
MH = MCH // 512          # 512-wide psum sub-slices per chunk

F32 = mybir.dt.float32
F16 = mybir.dt.float16

_NC_CACHE = {}


def _build(b_val: float, warm: int = 10):
    nc = bacc.Bacc("TRN2", target_bir_lowering=False, debug=False,
                   num_devices=N_CORES)

    tT = nc.dram_tensor("tT", [H, NSH], F16, kind="ExternalInput")
    aT = nc.dram_tensor("aT", [H, M_TOT], F16, kind="ExternalInput")
    W = nc.dram_tensor("W", [H, H], F16, kind="ExternalInput")
    # ut comes in as [H, 2] (two identical columns -> a 2-wide moving
    # operand; both result columns equal tvec).
    ut = nc.dram_tensor("ut", [H, 2], F16, kind="ExternalInput")
    ua = nc.dram_tensor("ua", [H, 1], F32, kind="ExternalInput")
    out = nc.dram_tensor("out", [NSH, M_TOT], F16, kind="ExternalOutput")

    # DRAM views with the k-tile index split out: row kt*128 + p.
    tT_v = tT.rearrange("(kt p) n -> p kt n", p=128)
    aT_v = aT.rearrange("(kt p) m -> p kt m", p=128)
    W_v = W.rearrange("(kt p) f -> p kt f", p=128)
    ut_v = ut.rearrange("(kt p) two -> p kt two", p=128)
    ua_v = ua.rearrange("(kt p) one -> p (kt one)", p=128)

    with TileContext(nc) as tc:
        with (
            tc.tile_pool(name="const", bufs=1) as cpool,
            tc.tile_pool(name="outbuf", bufs=6) as opool,
            tc.tile_pool(name="ps1", bufs=1, space="PSUM") as ps1pool,
            tc.tile_pool(name="ps2", bufs=4, space="PSUM") as ps2pool,
        ):
            w_sb = cpool.tile([128, KT, H], F16, tag="w")
            tT_sb = cpool.tile([128, KT, NSH], F16, tag="tT")
            at_sb = cpool.tile([128, KT, M_TOT], F16, tag="at")
            ua_sb = cpool.tile([128, KT], F32, tag="ua")
            ut_sb = cpool.tile([128, KT, 2], F16, tag="ut")

            # ---- load DMAs. k0's W/tT pair goes first on the two HWDGE
            # FIFOs so mm1 can start as soon as possible; later k-tiles
            # alternate FIFOs. ut/ua and the first at-chunk ride the
            # GpSimd SWDGE queue so they cost no sync/scalar issue slots.
            for k in range(KT):
                eng_w = nc.sync if k % 2 == 0 else nc.scalar
                eng_t = nc.scalar if k % 2 == 0 else nc.sync
                eng_w.dma_start(out=w_sb[:, k, :], in_=W_v[:, k, :])
                eng_t.dma_start(out=tT_sb[:, k, :], in_=tT_v[:, k, :])
            nc.gpsimd.dma_start(out=ut_sb[:], in_=ut_v[:])
            nc.gpsimd.dma_start(out=ua_sb[:], in_=ua_v[:])
            # at chunk 0 on SWDGE (needed ~20 us in); chunks 1-3 on sync
            # after the W/tT tiles.
            nc.gpsimd.dma_start(out=at_sb[:, :, 0:MCH],
                                in_=aT_v[:, :, 0:MCH])
            for c in range(1, MC):
                nc.sync.dma_start(
                    out=at_sb[:, :, c * MCH:(c + 1) * MCH],
                    in_=aT_v[:, :, c * MCH:(c + 1) * MCH],
                )

            # ---- PE warmup: the ~7.3us framework preamble leaves the PE
            # idle, so the HAM clock-gate sits at K=4/8 (half clock). A
            # short burst of dummy matmuls starts the busy window early so
            # the gate reaches 8/8 while mm1 streams. N=256 keeps each
            # warmup mm cheap; the count just has to bridge until mm1's
            # first k-tiles land. ----
            warm_w = cpool.tile([128, 256], F16, tag="warm_w")
            nc.gpsimd.memset(warm_w[:], 0.0)
            wps = ps1pool.tile([128, NSH], F32, tag="ps1_0", name="wps")
            for _ in range(warm):
                nc.tensor.matmul(wps[:, 0:256], warm_w[:, 0:128],
                                 warm_w[:], start=True, stop=True)

            # ---- mm1: TWt[h_out, n] = (T @ W)^T, + u_a bias on copy-out.
            # Two ho-half passes, k-outer over 4 PSUM banks each: a k step
            # only needs the W/tT k-tile k, so the PE starts as soon as the
            # first pair of DMAs lands; pass B reuses the resident W. ----
            twt_sb = cpool.tile([128, KT, NSH], F16, tag="twt")
            ps1 = [ps1pool.tile([128, NSH], F32, tag=f"ps1_{j}",
                                name=f"ps1_{j}")
                   for j in range(4)]
            for half in range(2):
                for k in range(KT):
                    for j in range(4):
                        ho = half * 4 + j
                        nc.tensor.matmul(
                            ps1[j][:],
                            w_sb[:, k, ho * 128:(ho + 1) * 128],
                            tT_sb[:, k, :],
                            start=(k == 0),
                            stop=(k == KT - 1),
                        )
                for j in range(4):
                    ho = half * 4 + j
                    # TWt[ho] = psum + u_a[ho-tile] (per-partition bias),
                    # cast to fp16 for mm2. Copies land after the final
                    # k row; alternate DVE/ACT to halve the serial latency
                    # gating mm2's start.
                    if j % 2 == 0:
                        nc.vector.tensor_scalar_add(
                            out=twt_sb[:, ho, :], in0=ps1[j][:],
                            scalar1=ua_sb[:, ho:ho + 1],
                        )
                    else:
                        nc.scalar.activation(
                            out=twt_sb[:, ho, :], in_=ps1[j][:],
                            func=mybir.ActivationFunctionType.Identity,
                            bias=ua_sb[:, ho:ho + 1],
                        )

            # ---- tvec[n] = T @ u_t + b: 32 tiny matmuls (ut is the 2-wide
            # moving operand; psum column 0 is tvec). ----
            tvec_sb = cpool.tile([128, NI], F32, tag="tvec")
            for ni in range(NI):
                psv = ps2pool.tile([128, 2], F32, tag="ps", name="psv")
                for k in range(KT):
                    nc.tensor.matmul(
                        psv[:],
                        tT_sb[:, k, ni * 128:(ni + 1) * 128],
                        ut_sb[:, k, :],
                        start=(k == 0),
                        stop=(k == KT - 1),
                    )
                nc.scalar.activation(
                    out=tvec_sb[:, ni:ni + 1], in_=psv[:, 0:1],
                    func=mybir.ActivationFunctionType.Identity,
                    bias=float(b_val),
                )

            # ---- mm2: S[n, m] = sum_k TWt[k]^T @ At[k], + tvec bias.
            # Output tiles store as fp16; copies alternate DVE/ACT and
            # store dma_starts alternate sync/scalar FIFOs. ----
            tile_idx = 0
            for c in range(MC):
                for ni in range(NI):
                    for h in range(MH):
                        ps = ps2pool.tile([128, 512], F32, tag="ps", name="ps")
                        for k in range(KT):
                            nc.tensor.matmul(
                                ps[:],
                                twt_sb[:, k, ni * 128:(ni + 1) * 128],
                                at_sb[:, k, c * MCH + h * 512:
                                      c * MCH + (h + 1) * 512],
                                start=(k == 0),
                                stop=(k == KT - 1),
                            )
                        o_sb = opool.tile([128, 512], F16, tag="o")
                        if tile_idx % 2 == 0:
                            nc.vector.tensor_scalar_add(
                                out=o_sb[:], in0=ps[:],
                                scalar1=tvec_sb[:, ni:ni + 1],
                            )
                            st_eng = nc.scalar
                        else:
                            nc.scalar.activation(
                                out=o_sb[:], in_=ps[:],
                                func=mybir.ActivationFunctionType.Identity,
                                bias=tvec_sb[:, ni:ni + 1],
                            )
                            st_eng = nc.sync
                        st_eng.dma_start(
                            out=out[ni * 128:(ni + 1) * 128,
                                    c * MCH + h * 512:c * MCH + (h + 1) * 512],
                            in_=o_sb[:],
                        )
                        tile_idx += 1

    nc.compile()
    return nc


def _get_nc(b_val: float):
    key = float(b_val)
    if key not in _NC_CACHE:
        _NC_CACHE[key] = _build(key)
    return _NC_CACHE[key]


def make_in_maps(target_spans, argument_spans, W, U, b):
    """Host-side layout prep: shard/transpose/cast the full inputs into the
    per-core input maps. Returns (in_maps, b_val)."""
    target_spans = np.asarray(target_spans, dtype=np.float32)
    argument_spans = np.asarray(argument_spans, dtype=np.float32)
    W = np.ascontiguousarray(np.asarray(W, dtype=np.float16))
    U = np.asarray(U, dtype=np.float32).reshape(2 * H, 1)
    b_val = float(np.asarray(b).reshape(-1)[0])

    tT = np.ascontiguousarray(target_spans.T.astype(np.float16))  # [H, N_TOT]
    aT = np.ascontiguousarray(argument_spans.T.astype(np.float16))  # [H, M_TOT]
    ut = np.ascontiguousarray(
        np.repeat(U[:H], 2, axis=1).astype(np.float16))  # [H, 2]
    ua = np.ascontiguousarray(U[H:])

    in_maps = [
        {
            "tT": np.ascontiguousarray(tT[:, i * NSH:(i + 1) * NSH]),
            "aT": aT,
            "W": W,
            "ut": ut,
            "ua": ua,
        }
        for i in range(N_CORES)
    ]
    return in_maps, b_val


def kernel(target_spans, argument_spans, W, U, b):
    in_maps, b_val = make_in_maps(target_spans, argument_spans, W, U, b)
    nc = _get_nc(b_val)
    res = run_bass_kernel_spmd(nc, in_maps, core_ids=list(range(N_CORES)))
    out = np.concatenate(
        [res.results[i]["out"] for i in range(N_CORES)], axis=0
    )
    return out.astype(np.float32, copy=False)


# revision 3
# speedup vs baseline: 1.1366x; 1.0912x over previous
"""Trainium2 Bass kernel for BiaffinePairing.

Computes S = (T @ W) @ A^T + T @ U[:H] + (A @ U[H:]).T + b  -> [4096, 4096] f32.

Strategy (8 NeuronCores, data-parallel over T's row dim n):
  - Host-side layout prep only (no math): transpose T and A so the
    contraction dim H=1024 lies on SBUF partitions; shard T^T's columns
    (the n dim) 8 ways; replicate A^T, W, and the U halves. Matmul inputs
    are pre-cast to fp16 on the host (~4e-4 relative error end-to-end).
    All DRAM-side operands are additionally permuted so that every DMA the
    kernel issues reads per-partition-contiguous lines (descriptor size is
    what sets DMA throughput, especially while the SDMA engines are still
    cold in the first ~15 us of the kernel).
  - Per core: mm1 computes TWt[h_out, n] = (T_shard @ W)^T accumulating in
    PSUM; the rank-1 term 1_n (x) (A @ u_a)^T folds in by adding u_a[h] as
    a per-partition bias on mm1's PSUM->SBUF copy (since
    (TW + 1 (x) u_a^T) @ A^T = TW@A^T + 1 (x) (A@u_a)^T).
  - tvec[n] = T_shard @ u_t + b via tiny matmuls; added as the per-partition
    bias on mm2's PSUM->SBUF copies.
  - mm2 computes S_shard[n, m] = sum_k TWt[k]^T @ At[k] over m-chunks.

Schedule notes (why the structure looks the way it is):
  - The PE stream rate is the wall: 320 N=512 matmuls run back-to-back at
    512 cycles each (s2s 216 ns at 2.4 GHz; 259 ns when the package power
    manager drops the cores to 2.0 GHz under sustained 8-core load). The
    optimization surface is the schedule fat around that stream.
  - Load order: W k0 / tT k0 split across both HWDGE FIFOs first (mm1's
    gate), then the remaining k-tiles alternating FIFOs, then the A
    chunks as 8 x 1 MB column-chunk DMAs (8 KB lines, chunk-major SBUF
    layout) alternating FIFOs. ut/ua ride the GpSimd SWDGE queue so they
    cost no HWDGE issue slots.
  - A tapered warmup burst of small matmuls keeps the PE HAM activity
    window busy from right after the framework preamble so the clock gate
    reaches 8/8 while mm1 streams, and bridges until k0 lands.
  - Outputs store as fp16 (upcast on host): halves store bytes and drain.
    Stores alternate FIFOs; the final tile stores in 4 slices so the
    end-of-kernel drain is short.
"""

import numpy as np

import concourse.bacc as bacc
import concourse.mybir as mybir
from concourse.tile import TileContext
from concourse.bass_utils import run_bass_kernel_spmd

H = 1024          # hidden dim (contraction)
N_TOT = 4096      # rows of target_spans
M_TOT = 4096      # rows of argument_spans
N_CORES = 8
NSH = N_TOT // N_CORES   # 512 n rows per core
KT = H // 128            # 8 contraction k-tiles
NI = NSH // 128          # 4 n-tiles of 128 per core
MCH = 512                # m column-chunk width (one PSUM tile per chunk)
MC = M_TOT // MCH        # 8 m-chunks

F32 = mybir.dt.float32
F16 = mybir.dt.float16

_NC_CACHE = {}


def _build(b_val: float, warm: int = 26):
    nc = bacc.Bacc("TRN2", target_bir_lowering=False, debug=False,
                   num_devices=N_CORES)

    # All inputs are host-permuted so partition p's data is contiguous in
    # DRAM (see make_in_maps): dim0 is the SBUF partition.
    tT = nc.dram_tensor("tT", [128, KT, NSH], F16, kind="ExternalInput")
    aT = nc.dram_tensor("aT", [128, MC, KT, MCH], F16, kind="ExternalInput")
    W = nc.dram_tensor("W", [128, KT, H], F16, kind="ExternalInput")
    ut = nc.dram_tensor("ut", [128, KT, 2], F16, kind="ExternalInput")
    ua = nc.dram_tensor("ua", [128, KT], F32, kind="ExternalInput")
    out = nc.dram_tensor("out", [NSH, M_TOT], F16, kind="ExternalOutput")

    with TileContext(nc) as tc:
        with (
            tc.tile_pool(name="const", bufs=1) as cpool,
            tc.tile_pool(name="outbuf", bufs=6) as opool,
            tc.tile_pool(name="ps1", bufs=1, space="PSUM") as ps1pool,
            tc.tile_pool(name="ps2", bufs=4, space="PSUM") as ps2pool,
        ):
            w_sb = cpool.tile([128, KT, H], F16, tag="w")
            tT_sb = cpool.tile([128, KT, NSH], F16, tag="tT")
            at_sb = cpool.tile([128, MC, KT, MCH], F16, tag="at")
            ua_sb = cpool.tile([128, KT], F32, tag="ua")
            ut_sb = cpool.tile([128, KT, 2], F16, tag="ut")

            # ---- load DMAs. k0's W/tT go first, each split across both
            # HWDGE FIFOs (halving first-tile latency on the cold SDMA
            # engines); later k-tiles alternate FIFOs; the A column-chunks
            # follow; ut/ua ride the SWDGE queue. ----
            nc.sync.dma_start(out=w_sb[0:64, 0, :], in_=W[0:64, 0, :])
            nc.scalar.dma_start(out=w_sb[64:128, 0, :], in_=W[64:128, 0, :])
            nc.sync.dma_start(out=tT_sb[0:64, 0, :], in_=tT[0:64, 0, :])
            nc.scalar.dma_start(out=tT_sb[64:128, 0, :], in_=tT[64:128, 0, :])
            nc.gpsimd.dma_start(out=ut_sb[:], in_=ut[:])
            nc.gpsimd.dma_start(out=ua_sb[:], in_=ua[:])
            for k in range(1, KT):
                eng_w = nc.sync if k % 2 == 0 else nc.scalar
                eng_t = nc.scalar if k % 2 == 0 else nc.sync
                eng_w.dma_start(out=w_sb[:, k, :], in_=W[:, k, :])
                eng_t.dma_start(out=tT_sb[:, k, :], in_=tT[:, k, :])
            for c in range(MC):
                eng = nc.sync if c % 2 == 0 else nc.scalar
                eng.dma_start(out=at_sb[:, c, :, :], in_=aT[:, c, :, :])

            # ---- PE warmup: the ~7.3us framework preamble leaves the PE
            # idle, so the HAM clock-gate sits at K=4/8 (half clock). A
            # burst of small dummy matmuls keeps the PE busy from the
            # earliest instant until mm1's first k-tile lands, so the gate
            # reaches 8/8 (full clock) as early as possible. ----
            warm_w = cpool.tile([128, 128], F16, tag="warm_w")
            nc.vector.memset(warm_w[:], 0.0)
            wps = ps1pool.tile([128, NSH], F32, tag="ps1_0", name="wps")
            for _ in range(warm):
                nc.tensor.matmul(wps[:, 0:128], warm_w[:], warm_w[:],
                                 start=True, stop=True)

            # ---- mm1: TWt[h_out, n] = (T @ W)^T, + u_a bias on copy-out.
            # Two ho-half passes, k-outer over 4 PSUM banks each: a k step
            # only needs the W/tT k-tile k, so the PE chases the loads. ----
            twt_sb = cpool.tile([128, KT, NSH], F16, tag="twt")
            ps1 = [ps1pool.tile([128, NSH], F32, tag=f"ps1_{j}",
                                name=f"ps1_{j}")
                   for j in range(4)]
            for half in range(2):
                for k in range(KT):
                    for j in range(4):
                        ho = half * 4 + j
                        nc.tensor.matmul(
                            ps1[j][:],
                            w_sb[:, k, ho * 128:(ho + 1) * 128],
                            tT_sb[:, k, :],
                            start=(k == 0),
                            stop=(k == KT - 1),
                        )
                for j in range(4):
                    ho = half * 4 + j
                    # TWt[ho] = psum + u_a[ho-tile] (per-partition bias),
                    # cast to fp16 for mm2; alternate DVE/ACT so the
                    # copies gating mm2's start aren't serialized.
                    if j % 2 == 0:
                        nc.vector.tensor_scalar_add(
                            out=twt_sb[:, ho, :], in0=ps1[j][:],
                            scalar1=ua_sb[:, ho:ho + 1],
                        )
                    else:
                        nc.scalar.activation(
                            out=twt_sb[:, ho, :], in_=ps1[j][:],
                            func=mybir.ActivationFunctionType.Identity,
                            bias=ua_sb[:, ho:ho + 1],
                        )

            # ---- tvec[n] = T @ u_t + b: 32 tiny matmuls (ut is the 2-wide
            # moving operand; psum column 0 is tvec). ----
            tvec_sb = cpool.tile([128, NI], F32, tag="tvec")
            for ni in range(NI):
                psv = ps2pool.tile([128, 2], F32, tag="ps", name="psv")
                for k in range(KT):
                    nc.tensor.matmul(
                        psv[:],
                        tT_sb[:, k, ni * 128:(ni + 1) * 128],
                        ut_sb[:, k, :],
                        start=(k == 0),
                        stop=(k == KT - 1),
                    )
                nc.scalar.activation(
                    out=tvec_sb[:, ni:ni + 1], in_=psv[:, 0:1],
                    func=mybir.ActivationFunctionType.Identity,
                    bias=float(b_val),
                )

            # ---- mm2: S[n, m] = sum_k TWt[k]^T @ At[k], + tvec bias.
            # Output tiles store as fp16; copies alternate DVE/ACT and
            # store dma_starts alternate FIFOs. The very last tile is
            # copied and stored in 4 column slices across both FIFOs so
            # the end-of-kernel drain after the final matmul is short. ----
            tile_idx = 0
            n_tiles = MC * NI
            for c in range(MC):
                for ni in range(NI):
                    ps = ps2pool.tile([128, 512], F32, tag="ps", name="ps")
                    for k in range(KT):
                        nc.tensor.matmul(
                            ps[:],
                            twt_sb[:, k, ni * 128:(ni + 1) * 128],
                            at_sb[:, c, k, :],
                            start=(k == 0),
                            stop=(k == KT - 1),
                        )
                    last = tile_idx == n_tiles - 1
                    o_sb = opool.tile([128, 512], F16, tag="o")
                    if not last:
                        if tile_idx % 2 == 0:
                            nc.vector.tensor_scalar_add(
                                out=o_sb[:], in0=ps[:],
                                scalar1=tvec_sb[:, ni:ni + 1],
                            )
                            st_eng = nc.scalar
                        else:
                            nc.scalar.activation(
                                out=o_sb[:], in_=ps[:],
                                func=mybir.ActivationFunctionType.Identity,
                                bias=tvec_sb[:, ni:ni + 1],
                            )
                            st_eng = nc.sync
                        st_eng.dma_start(
                            out=out[ni * 128:(ni + 1) * 128,
                                    c * MCH:(c + 1) * MCH],
                            in_=o_sb[:],
                        )
                    else:
                        for s in range(4):
                            sl = slice(s * 128, (s + 1) * 128)
                            if s % 2 == 0:
                                nc.vector.tensor_scalar_add(
                                    out=o_sb[:, sl], in0=ps[:, sl],
                                    scalar1=tvec_sb[:, ni:ni + 1],
                                )
                                st_eng = nc.scalar
                            else:
                                nc.scalar.activation(
                                    out=o_sb[:, sl], in_=ps[:, sl],
                                    func=mybir.ActivationFunctionType.Identity,
                                    bias=tvec_sb[:, ni:ni + 1],
                                )
                                st_eng = nc.sync
                            st_eng.dma_start(
                                out=out[ni * 128:(ni + 1) * 128,
                                        c * MCH + s * 128:
                                        c * MCH + (s + 1) * 128],
                                in_=o_sb[:, sl],
                            )
                    tile_idx += 1

    nc.compile()
    return nc


def _get_nc(b_val: float):
    key = float(b_val)
    if key not in _NC_CACHE:
        _NC_CACHE[key] = _build(key)
    return _NC_CACHE[key]


def make_in_maps(target_spans, argument_spans, W, U, b):
    """Host-side layout prep: shard/transpose/cast/permute the full inputs
    into the per-core input maps. Returns (in_maps, b_val).

    Permutations put SBUF partition p's data contiguous in DRAM:
      W    [128, KT, H]       W_perm[p, k, f]    = W[k*128+p, f]
      tT   [128, KT, NSH]     (per core shard)   = T[n0+n, k*128+p]^T
      aT   [128, MC, KT, MCH] chunk-major        = A[c*MCH+m, k*128+p]^T
    """
    target_spans = np.asarray(target_spans, dtype=np.float32)
    argument_spans = np.asarray(argument_spans, dtype=np.float32)
    W = np.asarray(W, dtype=np.float32)
    U = np.asarray(U, dtype=np.float32).reshape(2 * H, 1)
    b_val = float(np.asarray(b).reshape(-1)[0])

    # [H, X] -> [128, KT, X] with row k*128+p -> [p, k]
    def perm_kp(mat_hx):
        return np.ascontiguousarray(
            mat_hx.reshape(KT, 128, -1).transpose(1, 0, 2))

    W_p = perm_kp(W.astype(np.float16))                       # [128, KT, H]
    tT_full = target_spans.T.astype(np.float16)               # [H, N_TOT]
    aT_full = argument_spans.T.astype(np.float16)             # [H, M_TOT]
    # aT chunk-major: [128, MC, KT, MCH]
    aT_p = np.ascontiguousarray(
        aT_full.reshape(KT, 128, MC, MCH).transpose(1, 2, 0, 3))
    ut_p = perm_kp(np.repeat(U[:H], 2, axis=1).astype(np.float16))
    ua_p = np.ascontiguousarray(
        U[H:].reshape(KT, 128).transpose(1, 0))               # [128, KT]

    tT_p = perm_kp(tT_full)                                   # [128,KT,N_TOT]
    in_maps = [
        {
            "tT": np.ascontiguousarray(tT_p[:, :, i * NSH:(i + 1) * NSH]),
            "aT": aT_p,
            "W": W_p,
            "ut": ut_p,
            "ua": ua_p,
        }
        for i in range(N_CORES)
    ]
    return in_maps, b_val


def kernel(target_spans, argument_spans, W, U, b):
    in_maps, b_val = make_in_maps(target_spans, argument_spans, W, U, b)
    nc = _get_nc(b_val)
    res = run_bass_kernel_spmd(nc, in_maps, core_ids=list(range(N_CORES)))
    out = np.concatenate(
        [res.results[i]["out"] for i in range(N_CORES)], axis=0
    )
    return out.astype(np.float32, copy=False)
